# revision 1
# baseline (speedup 1.0000x reference)
"""LoRA Linear (y = x @ W^T + bias + x @ (B@A)^T) on 8 Trainium2 NeuronCores.

Strategy (column-parallel, per the out_features sharding):
  - Each core owns a 512-wide slice of out_features.
  - On device, the LoRA delta is folded into the weight once:
        W_eff^T = W_shard^T + A^T @ B_shard^T        (32 small matmuls)
    then the main GEMM runs as 64 token-tiles x 32 k-tiles of
    128x128x512 fp32r matmuls (fp22 multiply, fp32 accumulate) with the
    weight resident in SBUF and x streamed with 16KB contiguous DMA lines.
  - psum layout is [128 tokens, 512 out]; bias is added during PSUM
    eviction; output rows land directly in [tokens, out_shard] layout so
    the host-side gather is a plain concatenate.

Host-side work is layout only: pack x as [p, T, a, t] (so each token-tile
DMA is 128 partitions x 16KB contiguous), pre-transpose W/B slices, and
broadcast bias; then concatenate the 8 output shards.
"""

import numpy as np

B_DIM, S_DIM = 4, 2048
IN_F = 4096
OUT_F = 4096
RANK = 16
N_CORES = 8
O_SHARD = OUT_F // N_CORES          # 512
TOK = B_DIM * S_DIM                 # 8192
T_TILES = TOK // 128                # 64
K_TILES = IN_F // 128               # 32
N_XBUF = 4                          # x-tile pool bufs
N_XPREFETCH = 0                     # x tiles DMA'd ahead of the W stream
                                    # (prefetch ahead of W measured slower)

_CACHE = {}
LAST_RESULTS = None  # test harness introspection


def _build_nc():
    import concourse.mybir as mybir
    import concourse.tile as tile
    from concourse import bacc

    nc = bacc.Bacc("TRN2", target_bir_lowering=False)
    f32 = mybir.dt.float32
    f32r = mybir.dt.float32r

    x_d = nc.dram_tensor("x_re", (128, T_TILES, K_TILES, 128), f32r,
                         kind="ExternalInput")
    w_d = nc.dram_tensor("w_re", (128, K_TILES, O_SHARD), f32r,
                         kind="ExternalInput")
    a_d = nc.dram_tensor("a_t", (RANK, IN_F), f32r, kind="ExternalInput")
    bt_d = nc.dram_tensor("b_t", (RANK, O_SHARD), f32r, kind="ExternalInput")
    bias_d = nc.dram_tensor("bias_b", (128, O_SHARD), f32,
                            kind="ExternalInput")
    y_d = nc.dram_tensor("y", (TOK, O_SHARD), f32, kind="ExternalOutput")

    with tile.TileContext(nc) as tc:
        with (
            tc.tile_pool(name="wpool", bufs=1) as wpool,
            tc.tile_pool(name="const", bufs=1) as const,
            tc.tile_pool(name="xpool", bufs=N_XBUF) as xpool,
            tc.tile_pool(name="opool", bufs=3) as opool,
            tc.tile_pool(name="psum", bufs=4, space="PSUM") as psum_pool,
        ):
            a_sb = const.tile([RANK, IN_F], f32r)
            nc.sync.dma_start(a_sb[:], a_d[:])
            b_sb = const.tile([RANK, O_SHARD], f32r)
            nc.sync.dma_start(b_sb[:], bt_d[:])
            bias_sb = const.tile([128, O_SHARD], f32)
            nc.sync.dma_start(bias_sb[:], bias_d[:])

            # Prefetch the first token-tiles of x ahead of the weight
            # stream so the t=0 matmul chain can pace with W arrival.
            x_prefetch = []
            for t in range(N_XPREFETCH):
                x_sb = xpool.tile([128, K_TILES, 128], f32r)
                nc.sync.dma_start(x_sb[:], x_d[:, t, :, :])
                x_prefetch.append(x_sb)

            # Per-k-tile weight tiles so W DMA, the LoRA fold, and the main
            # matmuls pipeline instead of serializing on one big tile:
            # w_eff[a] = W^T[k-tile a] + A[:, a*128:(a+1)*128]^T @ B^T
            w_sb = []
            for a in range(K_TILES):
                w_t = wpool.tile([128, O_SHARD], f32r, tag=f"w{a}")
                nc.sync.dma_start(w_t[:], w_d[:, a, :])
                pd = psum_pool.tile([128, O_SHARD], f32)
                nc.tensor.matmul(
                    pd[:],
                    a_sb[:, a * 128:(a + 1) * 128],
                    b_sb[:],
                    start=True, stop=True,
                )
                nc.vector.tensor_add(w_t[:], w_t[:], pd[:])
                w_sb.append(w_t)

            # Main GEMM: psum[128t, 512o] = sum_a x_tile_a^T @ w_eff_a
            for t in range(T_TILES):
                if t < N_XPREFETCH:
                    x_sb = x_prefetch[t]
                else:
                    x_sb = xpool.tile([128, K_TILES, 128], f32r)
                    nc.sync.dma_start(x_sb[:], x_d[:, t, :, :])
                pt = psum_pool.tile([128, O_SHARD], f32)
                for a in range(K_TILES):
                    nc.tensor.matmul(
                        pt[:],
                        x_sb[:, a, :],
                        w_sb[a][:],
                        start=(a == 0), stop=(a == K_TILES - 1),
                    )
                o_sb = opool.tile([128, O_SHARD], f32)
                nc.vector.tensor_add(o_sb[:], pt[:], bias_sb[:])
                nc.sync.dma_start(y_d[t * 128:(t + 1) * 128, :], o_sb[:])

    nc.compile()
    return nc


def _pack_x(x):
    x2 = np.asarray(x, dtype=np.float32).reshape(TOK, IN_F)
    # x_re[p, T, a, t] = x2[T*128 + t, a*128 + p]
    xr = x2.reshape(T_TILES, 128, K_TILES, 128)      # (T, t, a, p)
    return np.ascontiguousarray(xr.transpose(3, 0, 2, 1))


def kernel(x, weight, A, B, bias):
    global LAST_RESULTS
    from concourse.bass_utils import run_bass_kernel_spmd

    if "nc" not in _CACHE:
        _CACHE["nc"] = _build_nc()
    nc = _CACHE["nc"]

    weight = np.asarray(weight, dtype=np.float32)
    A = np.asarray(A, dtype=np.float32)
    B = np.asarray(B, dtype=np.float32)
    bias = np.asarray(bias, dtype=np.float32)

    x_re = _pack_x(x)
    a_t = np.ascontiguousarray(A)

    in_maps = []
    for c in range(N_CORES):
        sl = slice(c * O_SHARD, (c + 1) * O_SHARD)
        w_s = weight[sl]                              # (512, 4096)
        # w_re[p, a, o] = w_s[o, a*128 + p]
        w_re = np.ascontiguousarray(
            w_s.T.reshape(K_TILES, 128, O_SHARD).transpose(1, 0, 2))
        b_t = np.ascontiguousarray(B[sl].T)           # (16, 512)
        bias_b = np.ascontiguousarray(
            np.broadcast_to(bias[sl], (128, O_SHARD)))
        in_maps.append({
            "x_re": x_re,
            "w_re": w_re,
            "a_t": a_t,
            "b_t": b_t,
            "bias_b": bias_b,
        })

    res = run_bass_kernel_spmd(nc, in_maps, core_ids=list(range(N_CORES)))
    LAST_RESULTS = res

    y = np.concatenate([res.results[c]["y"] for c in range(N_CORES)], axis=1)
    return y.reshape(B_DIM, S_DIM, OUT_F)



# revision 2
# speedup vs baseline: 1.1606x; 1.1606x over previous
"""LoRA Linear (y = x @ W^T + bias + x @ (B@A)^T) on 8 Trainium2 NeuronCores.

Strategy (column-parallel, per the out_features sharding):
  - Each core owns a 512-wide slice of out_features.
  - On device, the LoRA delta is folded into the weight once:
        W_eff^T = W_shard^T + A^T @ B_shard^T        (32 small matmuls)
    then the main GEMM runs as 64 token-tiles x 32 k-tiles of
    128x128x512 bf16 matmuls (fp32 accumulate) with the weight resident
    in SBUF and x streamed with 8KB contiguous DMA lines.
  - bf16 operands: matmul rate is the same as fp32r (1 cycle/row) but x
    DMA traffic halves, which takes HBM off the critical path
    (8 cores x 142MB fp32 reads ~= the whole compute time; bf16 cuts
    read traffic to ~71MB/core).
  - psum layout is [128 tokens, 512 out]; bias is added during PSUM
    eviction; output rows land directly in [tokens, out_shard] layout so
    the host-side gather is a plain concatenate.

Host-side work is layout only: pack x as [p, T, a, t] bf16 (so each
token-tile DMA is 128 partitions x 8KB contiguous), pre-transpose W/B
slices, and broadcast bias; then concatenate the 8 output shards.
"""

import numpy as np
import ml_dtypes

B_DIM, S_DIM = 4, 2048
IN_F = 4096
OUT_F = 4096
RANK = 16
N_CORES = 8
O_SHARD = OUT_F // N_CORES          # 512
TOK = B_DIM * S_DIM                 # 8192
T_TILES = TOK // 128                # 64
K_TILES = IN_F // 128               # 32
N_XBUF = 4                          # x-tile pool bufs

_CACHE = {}
LAST_RESULTS = None  # test harness introspection


def _build_nc():
    import concourse.mybir as mybir
    import concourse.tile as tile
    from concourse import bacc

    nc = bacc.Bacc("TRN2", target_bir_lowering=False)
    f32 = mybir.dt.float32
    bf16 = mybir.dt.bfloat16

    x_d = nc.dram_tensor("x_re", (128, T_TILES, K_TILES, 128), bf16,
                         kind="ExternalInput")
    w_d = nc.dram_tensor("w_re", (128, K_TILES, O_SHARD), bf16,
                         kind="ExternalInput")
    a_d = nc.dram_tensor("a_t", (RANK, IN_F), bf16, kind="ExternalInput")
    bt_d = nc.dram_tensor("b_t", (RANK, O_SHARD), bf16, kind="ExternalInput")
    bias_d = nc.dram_tensor("bias_b", (128, O_SHARD), f32,
                            kind="ExternalInput")
    y_d = nc.dram_tensor("y", (TOK, O_SHARD), f32, kind="ExternalOutput")

    with tile.TileContext(nc) as tc:
        with (
            tc.tile_pool(name="wpool", bufs=1) as wpool,
            tc.tile_pool(name="const", bufs=1) as const,
            tc.tile_pool(name="xpool", bufs=N_XBUF) as xpool,
            tc.tile_pool(name="opool", bufs=3) as opool,
            tc.tile_pool(name="psum", bufs=4, space="PSUM") as psum_pool,
        ):
            a_sb = const.tile([RANK, IN_F], bf16)
            nc.sync.dma_start(a_sb[:], a_d[:])
            b_sb = const.tile([RANK, O_SHARD], bf16)
            nc.sync.dma_start(b_sb[:], bt_d[:])
            bias_sb = const.tile([128, O_SHARD], f32)
            nc.sync.dma_start(bias_sb[:], bias_d[:])

            # Per-k-tile weight tiles so W DMA, the LoRA fold, and the main
            # matmuls pipeline instead of serializing on one big tile:
            # w_eff[a] = W^T[k-tile a] + A[:, a*128:(a+1)*128]^T @ B^T
            w_sb = []
            for a in range(K_TILES):
                w_t = wpool.tile([128, O_SHARD], bf16, tag=f"w{a}")
                nc.sync.dma_start(w_t[:], w_d[:, a, :])
                pd = psum_pool.tile([128, O_SHARD], f32)
                nc.tensor.matmul(
                    pd[:],
                    a_sb[:, a * 128:(a + 1) * 128],
                    b_sb[:],
                    start=True, stop=True,
                )
                nc.vector.tensor_add(w_t[:], w_t[:], pd[:])
                w_sb.append(w_t)

            # Main GEMM: psum[128t, 512o] = sum_a x_tile_a^T @ w_eff_a
            for t in range(T_TILES):
                x_sb = xpool.tile([128, K_TILES, 128], bf16)
                nc.sync.dma_start(x_sb[:], x_d[:, t, :, :])
                pt = psum_pool.tile([128, O_SHARD], f32)
                for a in range(K_TILES):
                    nc.tensor.matmul(
                        pt[:],
                        x_sb[:, a, :],
                        w_sb[a][:],
                        start=(a == 0), stop=(a == K_TILES - 1),
                    )
                o_sb = opool.tile([128, O_SHARD], f32)
                nc.vector.tensor_add(o_sb[:], pt[:], bias_sb[:])
                nc.sync.dma_start(y_d[t * 128:(t + 1) * 128, :], o_sb[:])

    nc.compile()
    return nc


def _pack_x(x):
    x2 = np.asarray(x, dtype=np.float32).reshape(TOK, IN_F)
    # x_re[p, T, a, t] = x2[T*128 + t, a*128 + p]
    xr = x2.reshape(T_TILES, 128, K_TILES, 128)      # (T, t, a, p)
    return np.ascontiguousarray(
        xr.transpose(3, 0, 2, 1).astype(ml_dtypes.bfloat16))


def kernel(x, weight, A, B, bias):
    global LAST_RESULTS
    from concourse.bass_utils import run_bass_kernel_spmd

    if "nc" not in _CACHE:
        _CACHE["nc"] = _build_nc()
    nc = _CACHE["nc"]

    weight = np.asarray(weight, dtype=np.float32)
    A = np.asarray(A, dtype=np.float32)
    B = np.asarray(B, dtype=np.float32)
    bias = np.asarray(bias, dtype=np.float32)

    x_re = _pack_x(x)
    a_t = np.ascontiguousarray(A.astype(ml_dtypes.bfloat16))

    in_maps = []
    for c in range(N_CORES):
        sl = slice(c * O_SHARD, (c + 1) * O_SHARD)
        w_s = weight[sl]                              # (512, 4096)
        # w_re[p, a, o] = w_s[o, a*128 + p]
        w_re = np.ascontiguousarray(
            w_s.T.reshape(K_TILES, 128, O_SHARD).transpose(1, 0, 2)
            .astype(ml_dtypes.bfloat16))
        b_t = np.ascontiguousarray(B[sl].T.astype(ml_dtypes.bfloat16))
        bias_b = np.ascontiguousarray(
            np.broadcast_to(bias[sl], (128, O_SHARD)))
        in_maps.append({
            "x_re": x_re,
            "w_re": w_re,
            "a_t": a_t,
            "b_t": b_t,
            "bias_b": bias_b,
        })

    res = run_bass_kernel_spmd(nc, in_maps, core_ids=list(range(N_CORES)))
    LAST_RESULTS = res

    y = np.concatenate([res.results[c]["y"] for c in range(N_CORES)], axis=1)
    return y.reshape(B_DIM, S_DIM, OUT_F)
